# revision 58
# baseline (speedup 1.0000x reference)
"""Trainium2 Bass kernel for nn_BagKQMClassModel.

Computation (per batch item b):
    K[b,n,m]   = exp(-d2/(2 s^2)),  d2 = |A[b,n] - C[m]|^2
    out_w[b,m] = (1/N) sum_n comp_w[m] * K^2
    y_w        = out_w / sum_m out_w
    probs      = y_w @ (y_v^2),  y_v = c_y rows normalized

Key transformations:
  * K^2 = exp(-d2/s^2): one exp per (b,n,m) element.
  * d2 folded into one fp16 matmul with augmented contraction (34 rows):
        rows 0-31: data a_d * c_d;  row 32: CT 1, AT -a2/2;
        row 33: CT -b2/2, AT 1
    so exp arg = (2/s^2) * g with no ACT bias.
  * probs = T[:, :10] / T[:, 10], T = sum_{n,m} K2 * W with
    W[m, :10] = comp_w[m] * c_y[m]^2 / |c_y[m]|^2, W[m, 10] = comp_w[m].
  * m and (b,n) are PERMUTED vs the reference layout so every input DMA
    is contiguous (partition p holds a contiguous DRAM block):
    m = 16p + r;  bn: p = 4t + n//32, j = n%32.  All downstream sums are
    permutation-invariant since c_y/comp_w use the same m-permutation.
  * mm1 runs 2-way ROW-TILED: each 1024-col PSUM window is produced by
    TWO CONCURRENT 512-col matmuls on PE row-tiles (0,0) and (64,0).
    Tile A (array rows 0-33) holds j-slices 0-15; tile B (rows 64-97)
    holds j-slices 16-31 (operands duplicated at SBUF partitions
    64-97).  Window w of chunk c = A j-slices 4w..4w+3 | B j-slices
    16+4w..16+4w+3 - a j-permutation of the baseline layout, which all
    downstream reductions are invariant to.  Halves mm1 wall time even
    with the PE HAM-throttled at 1.2 GHz (which it is: the PE duty
    cycle in the ACT-paced loop never un-throttles the clock gate).
  * K2 is written to a FLAT sbuf buffer (64 windows, 128KB/partition,
    no ring) so each ACTIVATE carries exactly one fused semaphore wait
    (the mm1 pair) - no standalone EVENT_SEMAPHORE on the ACT queue.
  * One PSUM tile S[11, 128] accumulates mm2 over all 64 (m-chunk,
    window) steps; the final per-batch output is one [11, 32, 4]
    free-dim reduce + an 11x32 transpose.
  * mm2 runs as fp8e4 DoubleRow over CHUNK PAIRS (stationary W8
    [128, 2, 11], moving r3 [128, 2, 128]) - half the mm2 instruction
    count and half the stream cycles.  W8 is pre-scaled by 2048 so the
    tiny comp_w values stay inside fp8e4's dynamic range (min subnormal
    2^-9); the scale cancels in the final T[:, :10] / T[:, 10].
    mm2 drains are deferred to s>=16 so the in-loop W8 build never
    stalls the in-order PE queue.
  * The 8:1 bag reduction batches SIX chunks into one DVE op per
    halving level, amortizing the ~151-cycle DVE op overhead.
  * CT chunks 2-15, AT windows 1-3, and the W build all run INSIDE the
    main loop in PE/DVE slack (deadline-scheduled quanta, <=2/step with
    at most 2 PE transposes + 1 mm1 pair per step: ~1.09us PE < 1.15us
    ACT cadence, so ACT stays the sole pacer).

Sharding: batch 256 -> 32 items per core across 8 cores; c_x/c_y/comp_w
replicated. No collectives (forward only).
"""

import numpy as np

import concourse.bacc as bacc
import concourse.mybir as mybir
import concourse.tile as tile
from concourse.bass_utils import run_bass_kernel_spmd
from concourse.masks import make_identity

NCORES = 8
BS, N, DX, DY, M = 256, 128, 32, 10, 2048
BPC = BS // NCORES      # 32 batch items per core
MB = M // 128           # 16 chunks of the component axis
KAUG = DX + 2           # 34 augmented contraction rows
NJ = 32                 # bn = 32 j-slices x 128 p
NJH = NJ // 2           # 16 j-slices per row-tile
WIN = 1024              # ACT window = one (chunk, window) step
TOT = 4 * MB            # 64 steps
GRP = 6                 # chunks per DVE reduce group
MM2_START = 17          # earliest step for mm2 drains (W8 ready by ~s15)
WSCL = 2048.0           # fp8 mm2 weight prescale (cancels in the division)
MIN_SIGMA = 1e-3
FP32 = mybir.dt.float32
FP16 = mybir.dt.float16
FP8 = mybir.dt.float8e4
AX = mybir.AxisListType
ALU = mybir.AluOpType
ACTF = mybir.ActivationFunctionType
DR = mybir.MatmulPerfMode.DoubleRow


def _body(tc, inp, cx, cy, cw_d, out_d, scale):
    nc = tc.nc
    from contextlib import ExitStack

    with ExitStack() as ctx:
        const = ctx.enter_context(tc.tile_pool(name="const", bufs=1))
        work = ctx.enter_context(tc.tile_pool(name="work", bufs=2))
        psum = ctx.enter_context(tc.tile_pool(name="psum", bufs=1, space="PSUM"))

        # ---- contiguous input DMAs ----------------------------------------
        # The sync queue is HWDGE (fast descriptor path); the prologue-
        # critical slices go there FIRST and SMALL so their completion
        # semaphores fire early: cx chunks 0-1, A j 0-3 and 16-19.
        # Nothing issues DMA from the scalar queue: that engine is the
        # ACT pacer.
        # the three prologue-critical DMAs go on three DIFFERENT queues so
        # their triggers (0.6us of queue time each) and ~1.7us completion
        # latencies overlap: cx 0-1 on sync, A j0-3 on gpsimd, A j16-19 on
        # scalar (all idle at this point; scalar's first real work is the
        # exp-table preload ~9us before the first window at ~14us).
        cx_nat = const.tile([128, MB, DX], FP32)    # m = 16p + r
        cx_r = cx.rearrange("(p r) d -> p r d", p=128)
        nc.sync.dma_start(out=cx_nat[:, 0:2, :], in_=cx_r[:, 0:2, :])
        A_nat = const.tile([128, NJ, DX], FP32)     # p = 4t + n//32, j = n%32
        inp_r = inp.rearrange("t (a j) d -> (t a) j d", a=4)
        nc.gpsimd.dma_start(out=A_nat[:, 0:4, :], in_=inp_r[:, 0:4, :])
        nc.scalar.dma_start(out=A_nat[:, 16:20, :], in_=inp_r[:, 16:20, :])
        nc.sync.dma_start(out=cx_nat[:, 2:16, :], in_=cx_r[:, 2:16, :])
        nc.sync.dma_start(out=A_nat[:, 4:8, :], in_=inp_r[:, 4:8, :])
        nc.sync.dma_start(out=A_nat[:, 20:24, :], in_=inp_r[:, 20:24, :])

        # identity + the pack fill columns are emitted BEFORE the gpsimd
        # DMA triggers: they gate the first PE transposes.
        ident16 = const.tile([128, 128], FP16)
        make_identity(nc, ident16)

        # preload the exp table set during the prologue DMA wait
        warm = const.tile([128, 1], FP32)
        nc.gpsimd.memset(warm, 0.0)
        warm2 = const.tile([128, 1], FP32)
        nc.scalar.activation(warm2, warm, ACTF.Exp, bias=0.0, scale=1.0)

        # packed fp16 transpose sources
        cx_pack = const.tile([128, MB, KAUG], FP16)  # [d x32, 1.0, -b2/2]
        A_pack = const.tile([128, NJ, KAUG], FP16)   # [d x32, -a2/2, 1.0]
        nc.gpsimd.memset(cx_pack[:, :, DX : DX + 1], 1.0)
        nc.gpsimd.memset(A_pack[:, :, DX + 1 : DX + 2], 1.0)

        cy_nat = const.tile([128, MB, DY], FP32)
        nc.gpsimd.dma_start(out=cy_nat, in_=cy.rearrange("(p r) d -> p r d", p=128))
        cw_nat = const.tile([128, MB], FP32)
        nc.gpsimd.dma_start(out=cw_nat, in_=cw_d.rearrange("(p r) -> p r", p=128))
        nc.gpsimd.dma_start(out=A_nat[:, 8:16, :], in_=inp_r[:, 8:16, :])
        nc.gpsimd.dma_start(out=A_nat[:, 24:32, :], in_=inp_r[:, 24:32, :])

        # transposed operands; the "b" variants live at SBUF partitions
        # 64-97 to feed PE row-tile (64, 0).
        CT16a = const.tile([128, MB, 128], FP16)
        CT16b = const.tile([128, MB, 128], FP16)
        AT16a = const.tile([128, NJH, 128], FP16)   # j-slices 0-15
        AT16b = const.tile([128, NJH, 128], FP16)   # j-slices 16-31
        W8 = const.tile([128, MB // 2, 2, 32], FP8)   # chunk-pair, padded to 32 cols
        nc.gpsimd.memset(W8, 0.0)
        S = psum.tile([32, 128], FP32, tag="S")
        K2r = const.tile([128, TOT, WIN], FP16)      # flat: no ring
        K2f = K2r.rearrange("p w f -> p (w f)")

        one3 = lambda t: t.rearrange("p (s o) -> p s o", o=1)

        # ---- prep helpers --------------------------------------------------
        def quant_chain(nat, pack, lo, hi, colh, split=None):
            # split=None: all 5 ops; split=1: mul+reduce; split=2: rest
            if split in (None, 1):
                # bufs=4: with 2, consecutive chains false-serialize on the
                # shared pool slots (WAR), which stalled the whole prologue
                sq = work.tile([128, NJ, DX], FP32, tag="sq", bufs=4)
                sqv = sq[:, 0 : hi - lo, :]
                nc.vector.tensor_mul(sqv, nat[:, lo:hi, :], nat[:, lo:hi, :])
                mh = work.tile([128, NJ], FP32, tag="mh", bufs=4)
                mhv = mh[:, 0 : hi - lo]
                nc.vector.tensor_reduce(out=one3(mhv), in_=sqv, axis=AX.X, op=ALU.add)
                quant_chain.saved = (sq, mh)
            if split in (None, 2):
                sq, mh = quant_chain.saved
                mhv = mh[:, 0 : hi - lo]
                # fused scale-by--0.5 + fp16 write of the aug column
                nc.vector.tensor_scalar(
                    out=pack[:, lo:hi, colh : colh + 1],
                    in0=one3(mhv),
                    scalar1=-0.5,
                    scalar2=None,
                    op0=ALU.mult,
                )
                nc.vector.tensor_copy(pack[:, lo:hi, 0:DX], nat[:, lo:hi, :])

        # transpose scratch: 4 slots per PSUM tile (pool-rotated, bufs=3)
        # so a batch of transposes needs only 1-2 wide copies per tile
        def trk_tile():
            return psum.tile([128, 4, 128], FP16, tag="trk", bufs=3, name="trk")

        def transpose_into(trk, slot, pack, src_idx, base):
            nc.tensor.transpose(
                trk[base : base + KAUG, slot, :], pack[:, src_idx, :], ident16
            )

        def copy_slots(trk, slot_lo, n, dst, dst_lo, base, use_scalar):
            cp = nc.scalar.copy if use_scalar else nc.vector.tensor_copy
            cp(
                dst[base : base + KAUG, dst_lo : dst_lo + n, :],
                trk[base : base + KAUG, slot_lo : slot_lo + n, :],
            )

        def transpose_one(pack, dst, src_idx, dst_idx, base, use_scalar):
            trk = trk_tile()
            transpose_into(trk, 0, pack, src_idx, base)
            copy_slots(trk, 0, 1, dst, dst_idx, base, use_scalar)

        def ct_dup(c):
            # duplicate chunk c's CT to partitions 64-97 for row-tile B
            nc.sync.dma_start(
                out=CT16b[64 : 64 + KAUG, c, :], in_=CT16a[0:KAUG, c, :]
            )

        def w_chain(split=None):
            if split in (None, 1):
                sqy = work.tile([128, MB, DY], FP32, tag="sqy")
                nc.vector.tensor_mul(sqy, cy_nat, cy_nat)
                ssum = work.tile([128, MB], FP32, tag="ssum")
                nc.vector.tensor_reduce(out=one3(ssum), in_=sqy, axis=AX.X, op=ALU.add)
                w_chain.saved = (sqy, ssum)
            if split in (None, 2):
                sqy, ssum = w_chain.saved
                rec = work.tile([128, MB], FP32, tag="rec")
                nc.vector.reciprocal(rec, ssum)
                facr = work.tile([128, MB], FP32, tag="facr")
                nc.vector.tensor_mul(facr, rec, cw_nat)
                w_chain.saved = (sqy, facr)
            if split in (None, 3):
                sqy, facr = w_chain.saved
                facr_b = one3(facr).broadcast_to([128, MB, DY])
                wtmp = work.tile([128, MB, DY], FP32, tag="wtmp")
                nc.vector.tensor_mul(wtmp, sqy, facr_b)
                w8f = W8.rearrange("p a h c -> p (a h) c")     # [128, 16, 32]
                nc.vector.tensor_scalar_mul(w8f[:, :, 0:DY], wtmp, WSCL)
                nc.vector.tensor_scalar_mul(w8f[:, :, DY : DY + 1], one3(cw_nat), WSCL)

        # ---- prologue ------------------------------------------------------
        # Critical chain to the first mm1 pair: cx(0:2) -> ct0/ct1 (both
        # bases via PE, no DMA latency), chainA(0:4) -> at0-3a,
        # chainB(16:20) -> at16-19b.  A-side copies on scalar (idle until
        # the first window), B-side on vector.
        quant_chain(cx_nat, cx_pack, 0, 2, DX + 1)
        trkc = trk_tile()
        transpose_into(trkc, 0, cx_pack, 0, 0)
        transpose_into(trkc, 1, cx_pack, 1, 0)
        copy_slots(trkc, 0, 2, CT16a, 0, 0, True)
        transpose_into(trkc, 2, cx_pack, 0, 64)
        transpose_into(trkc, 3, cx_pack, 1, 64)
        copy_slots(trkc, 2, 2, CT16b, 0, 64, True)
        quant_chain(A_nat, A_pack, 0, 4, DX)
        trka = trk_tile()
        for j in range(4):
            transpose_into(trka, j, A_pack, j, 0)
        copy_slots(trka, 0, 4, AT16a, 0, 0, True)
        quant_chain(A_nat, A_pack, 16, 20, DX)
        trkb = trk_tile()
        for j in range(16, 20):
            transpose_into(trkb, j - 16, A_pack, j, 64)
        copy_slots(trkb, 0, 4, AT16b, 0, 64, True)
        quant_chain(cx_nat, cx_pack, 2, MB, DX + 1)

        # deferred prep, deadline-ordered (executed in PE/DVE slack):
        #  - ct(c) must complete by step c-1 (chunk c first used at step c)
        #  - AT window 1 (j 4-7, 20-23) by step 15, window 2 by 31, 3 by 47
        #  - W8 by step MM2_START-2; chains before their transposes.
        def ct_q(c):
            def f():
                transpose_one(cx_pack, CT16a, c, c, 0, False)
                ct_dup(c)
            return f

        def ct_qpair(c):
            # both bases via PE (for early chunks where the DMA-dup's ~2us
            # completion latency would land after the chunk's first use)
            def f():
                # copies on SCALAR: the vector queue is congested with
                # chain quanta at steps 0-1, which stalled mm1-B(2) ~1.8us;
                # the first ACT windows are PE-paced during the ramp, so
                # the scalar queue has the slack
                trk = trk_tile()
                transpose_into(trk, 0, cx_pack, c, 0)
                transpose_into(trk, 1, cx_pack, c, 64)
                copy_slots(trk, 0, 1, CT16a, c, 0, True)
                copy_slots(trk, 1, 1, CT16b, c, 64, True)
            return f

        def ata_q(j):
            return lambda: transpose_one(A_pack, AT16a, j, j, 0, False)

        def atb_q(j):
            return lambda: transpose_one(A_pack, AT16b, j, j - 16, 64, False)

        def chain_q(lo, hi, split):
            return lambda: quant_chain(A_nat, A_pack, lo, hi, DX, split=split)

        def wc_q(split):
            return lambda: w_chain(split=split)

        sched = {s: [] for s in range(TOT)}
        # ct2/ct3 go PE-both-bases at steps 0-1 (the DMA-dup latency would
        # miss their deadlines); ct4+ use transpose+dup with a 2-step lead.
        pairs = [
            (ct_qpair(2), chain_q(4, 8, None)),
            (ct_qpair(3), chain_q(20, 24, None)),
            (ct_qpair(4), None), (ct_qpair(5), None),
            (ct_q(6), ata_q(4)), (ct_q(7), None),   # s5 = r1 step: light
            (ct_q(8), atb_q(20)), (ct_q(9), ata_q(5)),
            (ct_q(10), atb_q(21)), (ct_q(11), ata_q(6)),
            (ct_q(12), atb_q(22)), (ct_q(13), None),  # s11 = r1 step
            (ct_q(14), ata_q(7)), (ct_q(15), atb_q(23)),
            (wc_q(1), wc_q(2)), (wc_q(3), chain_q(8, 16, 1)),
        ]
        for s, (q1, q2) in enumerate(pairs):
            sched[s] = [q for q in (q1, q2) if q is not None]
        # steps 16+: 1/step, skipping the r1 group-boundary steps
        rest = [chain_q(8, 16, 2), chain_q(24, 32, 1), chain_q(24, 32, 2)]
        for j in range(8, 16):
            rest += [ata_q(j), atb_q(j + 16)]
        free_slots = [s for s in range(16, TOT) if s % GRP != GRP - 1]
        assert len(free_slots) >= len(rest)
        for slot, q in zip(free_slots, rest):
            sched[slot] = [q]

        AT16aF = AT16a.rearrange("p j c -> p (j c)")
        AT16bF = AT16b.rearrange("p j c -> p (j c)")

        # ---- main loop ----------------------------------------------------
        def emit_reduce(grp_base, nch):
            kv = K2f[:, grp_base : grp_base + nch * WIN].rearrange(
                "p (t f) -> p t f", t=nch
            )
            r1 = work.tile([128, GRP, 512], FP16, tag="r1")
            r1v = r1[:, 0:nch, :]
            nc.vector.tensor_add(r1v, kv[:, :, 0:512], kv[:, :, 512:1024])
            r2 = work.tile([128, GRP, 256], FP16, tag="r2")
            r2v = r2[:, 0:nch, :]
            nc.vector.tensor_add(r2v, r1v[:, :, 0:256], r1v[:, :, 256:512])
            # final fold writes fp8e4 DIRECTLY (no separate CAST op): the
            # mm2 moving operand quantization is unchanged, but ~0.5us of
            # DVE per group disappears
            r38 = work.tile([128, GRP, 128], FP8, tag="r38", bufs=4)
            nc.vector.tensor_add(
                r38[:, 0:nch, :], r2v[:, :, 0:128], r2v[:, :, 128:256]
            )
            return r38

        def emit_mm2(entry):
            # one PAIR (or final single) per call, so at most one mm2
            # instruction lands per loop step and the PE step time stays
            # under the ACT window cadence
            r38t, s0, i, nch = entry
            s_ = s0 + i
            pr = (s_ % MB) // 2
            if i + 1 < nch:
                nc.tensor.matmul(
                    S,
                    W8[:, pr, :, :],
                    r38t[:, i : i + 2, :],
                    start=(s_ == 0),
                    stop=(s_ + 1 == TOT - 1),
                    perf_mode=DR,
                )
            else:
                # odd single chunk: plain fp8 matmul against one half
                # of the chunk-pair weight block
                nc.tensor.matmul(
                    S,
                    W8[:, pr, s_ % 2, :],
                    r38t[:, i, :],
                    start=(s_ == 0),
                    stop=(s_ == TOT - 1),
                )
            return s_ + min(2, nch - i)

        # A few windows' exp runs on the DVE instead of ACT via the
        # Schraudolph bit trick: the fp16 bit pattern of 2^v is
        # int16(1024*v + 15360 - 43) for v in [-14, 0], so one fused
        # tensor_scalar (fp32 PSUM -> int16 write into the fp16 K2
        # buffer) computes exp(scale*g) = 2^(scale*log2(e)*g) to ~3%.
        # The sawtooth washes out in the reductions (measured rel err
        # 1.39e-3 vs the 2e-2 gate).  Each offload moves ~1.1us from the
        # ACT pacer onto DVE slack.
        SCHR = {21, 27, 33, 39, 45, 51, 57}
        A_schr = 1024.0 * scale * 1.4426950408889634
        B_schr = 15360.0 - 43.0
        K2i = K2r.bitcast(mybir.dt.int16)

        pending = []
        mm2_done = 0
        for s in range(TOT):
            w, c = divmod(s, MB)
            gw = psum.tile([128, WIN], FP32, tag="g", bufs=2, name="gw")
            # two concurrent row-tiled 512-col matmuls fill the window
            nc.tensor.matmul(
                gw[:, 0:512],
                CT16a[0:KAUG, c, :],
                AT16aF[0:KAUG, w * 512 : (w + 1) * 512],
                start=True,
                stop=True,
            )
            nc.tensor.matmul(
                gw[:, 512:1024],
                CT16b[64 : 64 + KAUG, c, :],
                AT16bF[64 : 64 + KAUG, w * 512 : (w + 1) * 512],
                start=True,
                stop=True,
            )
            if s in SCHR:
                # high priority: the static DVE queue order must place this
                # BEFORE the nearby tree ops, or the PSUM slot frees late
                # and mm1(s+2) stalls the ACT pipeline
                with tc.high_priority(offset=60):
                    nc.vector.tensor_scalar(
                        out=K2i[:, s, :],
                        in0=gw,
                        scalar1=A_schr,
                        scalar2=B_schr,
                        op0=ALU.mult,
                        op1=ALU.add,
                    )
            else:
                nc.scalar.activation(
                    K2r[:, s, :], gw, ACTF.Exp, bias=0.0, scale=scale
                )
            if s % GRP == GRP - 1 and s < GRP * (TOT // GRP):
                grp = s // GRP
                r38t = emit_reduce(WIN * (GRP * grp), GRP)
                for i in range(0, GRP, 2):
                    pending.append((r38t, GRP * grp, i, GRP))
            if s == 61:
                # tail pair reduces while windows 62-63 still exp
                pending.append((emit_reduce(WIN * 60, 2), 60, 0, 2))
            if s == 62:
                # single so the post-loop serial chain is as short as possible
                pending.append((emit_reduce(WIN * 62, 1), 62, 0, 1))
            if (
                pending
                and s >= MM2_START
                and pending[0][1] + pending[0][3] + 2 <= s
            ):
                mm2_done = emit_mm2(pending.pop(0))
            for fn in sched[s]:
                fn()
        # tail: final window
        pending.append((emit_reduce(WIN * 63, 1), 63, 0, 1))
        for entry in pending:
            mm2_done = emit_mm2(entry)
        assert mm2_done == TOT

        # ---- epilogue: T = reduce(S); the tiny normalization/transpose
        # runs on the host (it is part of kernel(), not the reference).
        Tred = const.tile([DY + 1, BPC], FP32)
        nc.vector.tensor_reduce(
            out=Tred.rearrange("p (t o) -> p t o", o=1),
            in_=S[0 : DY + 1, :].rearrange("p (t f) -> p t f", f=4),
            axis=AX.X,
            op=ALU.add,
        )
        nc.sync.dma_start(out=out_d, in_=Tred)




def build_program(scale):
    nc = bacc.Bacc(
        "TRN2",
        target_bir_lowering=False,
        debug=False,
        enable_asserts=False,
        num_devices=NCORES,
    )
    inp = nc.dram_tensor("inputs", [BPC, N, DX], FP32, kind="ExternalInput").ap()
    cx = nc.dram_tensor("c_x", [M, DX], FP32, kind="ExternalInput").ap()
    cy = nc.dram_tensor("c_y", [M, DY], FP32, kind="ExternalInput").ap()
    cw = nc.dram_tensor("comp_w", [M], FP32, kind="ExternalInput").ap()
    out = nc.dram_tensor("out", [DY + 1, BPC], FP32, kind="ExternalOutput").ap()
    with tile.TileContext(nc) as tc:
        _body(tc, inp, cx, cy, cw, out, scale)
    nc.compile()
    return nc


_PROGRAM_CACHE: dict = {}


def _get_program(scale):
    nc = _PROGRAM_CACHE.get(scale)
    if nc is None:
        nc = build_program(scale)
        _PROGRAM_CACHE[scale] = nc
    return nc


def make_in_maps(inputs, c_x, c_y, comp_w):
    shards = np.ascontiguousarray(inputs.reshape(NCORES, BPC, N, DX))
    return [
        {
            "inputs": shards[i],
            "c_x": np.ascontiguousarray(c_x),
            "c_y": np.ascontiguousarray(c_y),
            "comp_w": np.ascontiguousarray(comp_w),
        }
        for i in range(NCORES)
    ]


def scale_from_sigma(sigma) -> float:
    s = max(float(np.asarray(sigma, dtype=np.float64)), MIN_SIGMA)
    return float(2.0 / (s * s))


def kernel(inputs, sigma, c_x, c_y, comp_w, _run_kwargs=None):
    nc = _get_program(scale_from_sigma(sigma))
    in_maps = make_in_maps(inputs, c_x, c_y, comp_w)
    res = run_bass_kernel_spmd(
        nc, in_maps, core_ids=list(range(NCORES)), **(_run_kwargs or {})
    )
    # T is [11, BPC] per core: rows 0-9 = unnormalized probs, row 10 = the
    # normalizer.  Finish the division + transpose here (tiny).
    T = np.stack([res.results[i]["out"] for i in range(NCORES)])  # [NC, 11, BPC]
    out = (T[:, :DY, :] / T[:, DY : DY + 1, :]).transpose(0, 2, 1)
    return np.ascontiguousarray(out.reshape(BS, DY)).astype(np.float32)


# revision 59
# speedup vs baseline: 1.0316x; 1.0316x over previous
"""Trainium2 Bass kernel for nn_BagKQMClassModel.

Computation (per batch item b):
    K[b,n,m]   = exp(-d2/(2 s^2)),  d2 = |A[b,n] - C[m]|^2
    out_w[b,m] = (1/N) sum_n comp_w[m] * K^2
    y_w        = out_w / sum_m out_w
    probs      = y_w @ (y_v^2),  y_v = c_y rows normalized

Key transformations:
  * K^2 = exp(-d2/s^2): one exp per (b,n,m) element.
  * d2 folded into one fp16 matmul with augmented contraction (34 rows):
        rows 0-31: data a_d * c_d;  row 32: CT 1, AT -a2/2;
        row 33: CT -b2/2, AT 1
    so exp arg = (2/s^2) * g with no ACT bias.
  * probs = T[:, :10] / T[:, 10], T = sum_{n,m} K2 * W with
    W[m, :10] = comp_w[m] * c_y[m]^2 / |c_y[m]|^2, W[m, 10] = comp_w[m].
  * m and (b,n) are PERMUTED vs the reference layout so every input DMA
    is contiguous (partition p holds a contiguous DRAM block):
    m = 16p + r;  bn: p = 4t + n//32, j = n%32.  All downstream sums are
    permutation-invariant since c_y/comp_w use the same m-permutation.
  * mm1 runs 2-way ROW-TILED: each 1024-col PSUM window is produced by
    TWO CONCURRENT 512-col matmuls on PE row-tiles (0,0) and (64,0).
    Tile A (array rows 0-33) holds j-slices 0-15; tile B (rows 64-97)
    holds j-slices 16-31 (operands duplicated at SBUF partitions
    64-97).  Window w of chunk c = A j-slices 4w..4w+3 | B j-slices
    16+4w..16+4w+3 - a j-permutation of the baseline layout, which all
    downstream reductions are invariant to.  Halves mm1 wall time even
    with the PE HAM-throttled at 1.2 GHz (which it is: the PE duty
    cycle in the ACT-paced loop never un-throttles the clock gate).
  * K2 is written to a FLAT sbuf buffer (64 windows, 128KB/partition,
    no ring) so each ACTIVATE carries exactly one fused semaphore wait
    (the mm1 pair) - no standalone EVENT_SEMAPHORE on the ACT queue.
  * One PSUM tile S[11, 128] accumulates mm2 over all 64 (m-chunk,
    window) steps; the final per-batch output is one [11, 32, 4]
    free-dim reduce + an 11x32 transpose.
  * mm2 runs as fp8e4 DoubleRow over CHUNK PAIRS (stationary W8
    [128, 2, 11], moving r3 [128, 2, 128]) - half the mm2 instruction
    count and half the stream cycles.  W8 is pre-scaled by 2048 so the
    tiny comp_w values stay inside fp8e4's dynamic range (min subnormal
    2^-9); the scale cancels in the final T[:, :10] / T[:, 10].
    mm2 drains are deferred to s>=16 so the in-loop W8 build never
    stalls the in-order PE queue.
  * The 8:1 bag reduction batches SIX chunks into one DVE op per
    halving level, amortizing the ~151-cycle DVE op overhead.
  * CT chunks 2-15, AT windows 1-3, and the W build all run INSIDE the
    main loop in PE/DVE slack (deadline-scheduled quanta, <=2/step with
    at most 2 PE transposes + 1 mm1 pair per step: ~1.09us PE < 1.15us
    ACT cadence, so ACT stays the sole pacer).

Sharding: batch 256 -> 32 items per core across 8 cores; c_x/c_y/comp_w
replicated. No collectives (forward only).
"""

import numpy as np

import concourse.bacc as bacc
import concourse.mybir as mybir
import concourse.tile as tile
from concourse.bass_utils import run_bass_kernel_spmd
from concourse.masks import make_identity

NCORES = 8
BS, N, DX, DY, M = 256, 128, 32, 10, 2048
BPC = BS // NCORES      # 32 batch items per core
MB = M // 128           # 16 chunks of the component axis
KAUG = DX + 2           # 34 augmented contraction rows
NJ = 32                 # bn = 32 j-slices x 128 p
NJH = NJ // 2           # 16 j-slices per row-tile
WIN = 1024              # ACT window = one (chunk, window) step
TOT = 4 * MB            # 64 steps
GRP = 6                 # chunks per DVE reduce group
MM2_START = 17          # earliest step for mm2 drains (W8 ready by ~s15)
WSCL = 2048.0           # fp8 mm2 weight prescale (cancels in the division)
MIN_SIGMA = 1e-3
FP32 = mybir.dt.float32
FP16 = mybir.dt.float16
FP8 = mybir.dt.float8e4
AX = mybir.AxisListType
ALU = mybir.AluOpType
ACTF = mybir.ActivationFunctionType
DR = mybir.MatmulPerfMode.DoubleRow


def _body(tc, inp, cx, cy, cw_d, out_d, scale):
    nc = tc.nc
    from contextlib import ExitStack

    with ExitStack() as ctx:
        const = ctx.enter_context(tc.tile_pool(name="const", bufs=1))
        work = ctx.enter_context(tc.tile_pool(name="work", bufs=2))
        psum = ctx.enter_context(tc.tile_pool(name="psum", bufs=1, space="PSUM"))

        # ---- contiguous input DMAs ----------------------------------------
        # The sync queue is HWDGE (fast descriptor path); the prologue-
        # critical slices go there FIRST and SMALL so their completion
        # semaphores fire early: cx chunks 0-1, A j 0-3 and 16-19.
        # Nothing issues DMA from the scalar queue: that engine is the
        # ACT pacer.
        # the three prologue-critical DMAs go on three DIFFERENT queues so
        # their triggers (0.6us of queue time each) and ~1.7us completion
        # latencies overlap: cx 0-1 on sync, A j0-3 on gpsimd, A j16-19 on
        # scalar (all idle at this point; scalar's first real work is the
        # exp-table preload ~9us before the first window at ~14us).
        cx_nat = const.tile([128, MB, DX], FP32)    # m = 16p + r
        cx_r = cx.rearrange("(p r) d -> p r d", p=128)
        nc.sync.dma_start(out=cx_nat[:, 0:2, :], in_=cx_r[:, 0:2, :])
        A_nat = const.tile([128, NJ, DX], FP32)     # p = 4t + n//32, j = n%32
        inp_r = inp.rearrange("t (a j) d -> (t a) j d", a=4)
        nc.gpsimd.dma_start(out=A_nat[:, 0:4, :], in_=inp_r[:, 0:4, :])
        nc.scalar.dma_start(out=A_nat[:, 16:20, :], in_=inp_r[:, 16:20, :])
        nc.sync.dma_start(out=cx_nat[:, 2:16, :], in_=cx_r[:, 2:16, :])
        nc.sync.dma_start(out=A_nat[:, 4:8, :], in_=inp_r[:, 4:8, :])
        nc.sync.dma_start(out=A_nat[:, 20:24, :], in_=inp_r[:, 20:24, :])

        # identity + the pack fill columns are emitted BEFORE the gpsimd
        # DMA triggers: they gate the first PE transposes.
        ident16 = const.tile([128, 128], FP16)
        make_identity(nc, ident16)

        # preload the exp table set during the prologue DMA wait
        warm = const.tile([128, 1], FP32)
        nc.gpsimd.memset(warm, 0.0)
        warm2 = const.tile([128, 1], FP32)
        nc.scalar.activation(warm2, warm, ACTF.Exp, bias=0.0, scale=1.0)

        # packed fp16 transpose sources
        cx_pack = const.tile([128, MB, KAUG], FP16)  # [d x32, 1.0, -b2/2]
        A_pack = const.tile([128, NJ, KAUG], FP16)   # [d x32, -a2/2, 1.0]
        nc.gpsimd.memset(cx_pack[:, :, DX : DX + 1], 1.0)
        nc.gpsimd.memset(A_pack[:, :, DX + 1 : DX + 2], 1.0)

        cy_nat = const.tile([128, MB, DY], FP32)
        nc.gpsimd.dma_start(out=cy_nat, in_=cy.rearrange("(p r) d -> p r d", p=128))
        cw_nat = const.tile([128, MB], FP32)
        nc.gpsimd.dma_start(out=cw_nat, in_=cw_d.rearrange("(p r) -> p r", p=128))
        nc.gpsimd.dma_start(out=A_nat[:, 8:16, :], in_=inp_r[:, 8:16, :])
        nc.gpsimd.dma_start(out=A_nat[:, 24:32, :], in_=inp_r[:, 24:32, :])

        # transposed operands; the "b" variants live at SBUF partitions
        # 64-97 to feed PE row-tile (64, 0).
        CT16a = const.tile([128, MB, 128], FP16)
        CT16b = const.tile([128, MB, 128], FP16)
        AT16a = const.tile([128, NJH, 128], FP16)   # j-slices 0-15
        AT16b = const.tile([128, NJH, 128], FP16)   # j-slices 16-31
        W8 = const.tile([128, MB // 2, 2, 32], FP8)   # chunk-pair, padded to 32 cols
        nc.gpsimd.memset(W8, 0.0)
        S = psum.tile([32, 128], FP32, tag="S")
        K2r = const.tile([128, TOT, WIN], FP16)      # flat: no ring
        K2f = K2r.rearrange("p w f -> p (w f)")

        one3 = lambda t: t.rearrange("p (s o) -> p s o", o=1)

        # ---- prep helpers --------------------------------------------------
        def quant_chain(nat, pack, lo, hi, colh, split=None):
            # split=None: all 5 ops; split=1: mul+reduce; split=2: rest
            if split in (None, 1):
                # bufs=4: with 2, consecutive chains false-serialize on the
                # shared pool slots (WAR), which stalled the whole prologue
                sq = work.tile([128, NJ, DX], FP32, tag="sq", bufs=4)
                sqv = sq[:, 0 : hi - lo, :]
                nc.vector.tensor_mul(sqv, nat[:, lo:hi, :], nat[:, lo:hi, :])
                mh = work.tile([128, NJ], FP32, tag="mh", bufs=4)
                mhv = mh[:, 0 : hi - lo]
                nc.vector.tensor_reduce(out=one3(mhv), in_=sqv, axis=AX.X, op=ALU.add)
                quant_chain.saved = (sq, mh)
            if split in (None, 2):
                sq, mh = quant_chain.saved
                mhv = mh[:, 0 : hi - lo]
                # fused scale-by--0.5 + fp16 write of the aug column
                nc.vector.tensor_scalar(
                    out=pack[:, lo:hi, colh : colh + 1],
                    in0=one3(mhv),
                    scalar1=-0.5,
                    scalar2=None,
                    op0=ALU.mult,
                )
                nc.vector.tensor_copy(pack[:, lo:hi, 0:DX], nat[:, lo:hi, :])

        # transpose scratch: 4 slots per PSUM tile (pool-rotated, bufs=3)
        # so a batch of transposes needs only 1-2 wide copies per tile
        def trk_tile():
            return psum.tile([128, 4, 128], FP16, tag="trk", bufs=3, name="trk")

        def transpose_into(trk, slot, pack, src_idx, base):
            nc.tensor.transpose(
                trk[base : base + KAUG, slot, :], pack[:, src_idx, :], ident16
            )

        def copy_slots(trk, slot_lo, n, dst, dst_lo, base, use_scalar):
            cp = nc.scalar.copy if use_scalar else nc.vector.tensor_copy
            cp(
                dst[base : base + KAUG, dst_lo : dst_lo + n, :],
                trk[base : base + KAUG, slot_lo : slot_lo + n, :],
            )

        def transpose_one(pack, dst, src_idx, dst_idx, base, use_scalar):
            trk = trk_tile()
            transpose_into(trk, 0, pack, src_idx, base)
            copy_slots(trk, 0, 1, dst, dst_idx, base, use_scalar)

        def ct_dup(c):
            # duplicate chunk c's CT to partitions 64-97 for row-tile B
            nc.sync.dma_start(
                out=CT16b[64 : 64 + KAUG, c, :], in_=CT16a[0:KAUG, c, :]
            )

        def w_chain(split=None):
            if split in (None, 1):
                sqy = work.tile([128, MB, DY], FP32, tag="sqy")
                nc.vector.tensor_mul(sqy, cy_nat, cy_nat)
                ssum = work.tile([128, MB], FP32, tag="ssum")
                nc.vector.tensor_reduce(out=one3(ssum), in_=sqy, axis=AX.X, op=ALU.add)
                w_chain.saved = (sqy, ssum)
            if split in (None, 2):
                sqy, ssum = w_chain.saved
                rec = work.tile([128, MB], FP32, tag="rec")
                nc.vector.reciprocal(rec, ssum)
                facr = work.tile([128, MB], FP32, tag="facr")
                nc.vector.tensor_mul(facr, rec, cw_nat)
                w_chain.saved = (sqy, facr)
            if split in (None, 3):
                sqy, facr = w_chain.saved
                facr_b = one3(facr).broadcast_to([128, MB, DY])
                wtmp = work.tile([128, MB, DY], FP32, tag="wtmp")
                nc.vector.tensor_mul(wtmp, sqy, facr_b)
                w8f = W8.rearrange("p a h c -> p (a h) c")     # [128, 16, 32]
                nc.vector.tensor_scalar_mul(w8f[:, :, 0:DY], wtmp, WSCL)
                nc.vector.tensor_scalar_mul(w8f[:, :, DY : DY + 1], one3(cw_nat), WSCL)

        # ---- prologue ------------------------------------------------------
        # Critical chain to the first mm1 pair: cx(0:2) -> ct0/ct1 (both
        # bases via PE, no DMA latency), chainA(0:4) -> at0-3a,
        # chainB(16:20) -> at16-19b.  A-side copies on scalar (idle until
        # the first window), B-side on vector.
        quant_chain(cx_nat, cx_pack, 0, 2, DX + 1)
        trkc = trk_tile()
        transpose_into(trkc, 0, cx_pack, 0, 0)
        transpose_into(trkc, 1, cx_pack, 1, 0)
        copy_slots(trkc, 0, 2, CT16a, 0, 0, True)
        transpose_into(trkc, 2, cx_pack, 0, 64)
        transpose_into(trkc, 3, cx_pack, 1, 64)
        copy_slots(trkc, 2, 2, CT16b, 0, 64, True)
        quant_chain(A_nat, A_pack, 0, 4, DX)
        trka = trk_tile()
        for j in range(4):
            transpose_into(trka, j, A_pack, j, 0)
        copy_slots(trka, 0, 4, AT16a, 0, 0, True)
        quant_chain(A_nat, A_pack, 16, 20, DX)
        trkb = trk_tile()
        for j in range(16, 20):
            transpose_into(trkb, j - 16, A_pack, j, 64)
        copy_slots(trkb, 0, 4, AT16b, 0, 64, True)
        quant_chain(cx_nat, cx_pack, 2, MB, DX + 1)

        # deferred prep, deadline-ordered (executed in PE/DVE slack):
        #  - ct(c) must complete by step c-1 (chunk c first used at step c)
        #  - AT window 1 (j 4-7, 20-23) by step 15, window 2 by 31, 3 by 47
        #  - W8 by step MM2_START-2; chains before their transposes.
        def ct_q(c):
            def f():
                transpose_one(cx_pack, CT16a, c, c, 0, False)
                ct_dup(c)
            return f

        def ct_qpair(c):
            # both bases via PE (for early chunks where the DMA-dup's ~2us
            # completion latency would land after the chunk's first use)
            def f():
                trk = trk_tile()
                transpose_into(trk, 0, cx_pack, c, 0)
                transpose_into(trk, 1, cx_pack, c, 64)
                copy_slots(trk, 0, 1, CT16a, c, 0, False)
                copy_slots(trk, 1, 1, CT16b, c, 64, False)
            return f

        def ata_q(j):
            return lambda: transpose_one(A_pack, AT16a, j, j, 0, False)

        def atb_q(j):
            return lambda: transpose_one(A_pack, AT16b, j, j - 16, 64, False)

        def chain_q(lo, hi, split):
            return lambda: quant_chain(A_nat, A_pack, lo, hi, DX, split=split)

        def wc_q(split):
            return lambda: w_chain(split=split)

        sched = {s: [] for s in range(TOT)}
        # ct2/ct3 go PE-both-bases at steps 0-1 (the DMA-dup latency would
        # miss their deadlines); ct4+ use transpose+dup with a 2-step lead.
        pairs = [
            (ct_qpair(2), chain_q(4, 8, None)),
            (ct_qpair(3), chain_q(20, 24, None)),
            (ct_qpair(4), None), (ct_qpair(5), None),
            (ct_q(6), ata_q(4)), (ct_q(7), None),   # s5 = r1 step: light
            (ct_q(8), atb_q(20)), (ct_q(9), ata_q(5)),
            (ct_q(10), atb_q(21)), (ct_q(11), ata_q(6)),
            (ct_q(12), atb_q(22)), (ct_q(13), None),  # s11 = r1 step
            (ct_q(14), ata_q(7)), (ct_q(15), atb_q(23)),
            (wc_q(1), wc_q(2)), (wc_q(3), chain_q(8, 16, 1)),
        ]
        for s, (q1, q2) in enumerate(pairs):
            sched[s] = [q for q in (q1, q2) if q is not None]
        # steps 16+: 1/step, skipping the r1 group-boundary steps
        rest = [chain_q(8, 16, 2), chain_q(24, 32, 1), chain_q(24, 32, 2)]
        for j in range(8, 16):
            rest += [ata_q(j), atb_q(j + 16)]
        free_slots = [s for s in range(16, TOT) if s % GRP != GRP - 1]
        assert len(free_slots) >= len(rest)
        for slot, q in zip(free_slots, rest):
            sched[slot] = [q]

        AT16aF = AT16a.rearrange("p j c -> p (j c)")
        AT16bF = AT16b.rearrange("p j c -> p (j c)")

        # ---- main loop ----------------------------------------------------
        def emit_reduce(grp_base, nch):
            kv = K2f[:, grp_base : grp_base + nch * WIN].rearrange(
                "p (t f) -> p t f", t=nch
            )
            r1 = work.tile([128, GRP, 512], FP16, tag="r1")
            r1v = r1[:, 0:nch, :]
            nc.vector.tensor_add(r1v, kv[:, :, 0:512], kv[:, :, 512:1024])
            r2 = work.tile([128, GRP, 256], FP16, tag="r2")
            r2v = r2[:, 0:nch, :]
            nc.vector.tensor_add(r2v, r1v[:, :, 0:256], r1v[:, :, 256:512])
            # final fold writes fp8e4 DIRECTLY (no separate CAST op): the
            # mm2 moving operand quantization is unchanged, but ~0.5us of
            # DVE per group disappears
            r38 = work.tile([128, GRP, 128], FP8, tag="r38", bufs=4)
            nc.vector.tensor_add(
                r38[:, 0:nch, :], r2v[:, :, 0:128], r2v[:, :, 128:256]
            )
            return r38

        def emit_mm2(entry):
            # one PAIR (or final single) per call, so at most one mm2
            # instruction lands per loop step and the PE step time stays
            # under the ACT window cadence
            r38t, s0, i, nch = entry
            s_ = s0 + i
            pr = (s_ % MB) // 2
            if i + 1 < nch:
                nc.tensor.matmul(
                    S,
                    W8[:, pr, :, :],
                    r38t[:, i : i + 2, :],
                    start=(s_ == 0),
                    stop=(s_ + 1 == TOT - 1),
                    perf_mode=DR,
                )
            else:
                # odd single chunk: plain fp8 matmul against one half
                # of the chunk-pair weight block
                nc.tensor.matmul(
                    S,
                    W8[:, pr, s_ % 2, :],
                    r38t[:, i, :],
                    start=(s_ == 0),
                    stop=(s_ == TOT - 1),
                )
            return s_ + min(2, nch - i)

        # A few windows' exp runs on the DVE instead of ACT via the
        # Schraudolph bit trick: the fp16 bit pattern of 2^v is
        # int16(1024*v + 15360 - 43) for v in [-14, 0], so one fused
        # tensor_scalar (fp32 PSUM -> int16 write into the fp16 K2
        # buffer) computes exp(scale*g) = 2^(scale*log2(e)*g) to ~3%.
        # The sawtooth washes out in the reductions (measured rel err
        # 1.39e-3 vs the 2e-2 gate).  Each offload moves ~1.1us from the
        # ACT pacer onto DVE slack.
        SCHR = {21, 27, 33, 39, 45, 51, 57}
        A_schr = 1024.0 * scale * 1.4426950408889634
        B_schr = 15360.0 - 43.0
        K2i = K2r.bitcast(mybir.dt.int16)

        pending = []
        mm2_done = 0
        for s in range(TOT):
            w, c = divmod(s, MB)
            gw = psum.tile([128, WIN], FP32, tag="g", bufs=2, name="gw")
            # two concurrent row-tiled 512-col matmuls fill the window
            nc.tensor.matmul(
                gw[:, 0:512],
                CT16a[0:KAUG, c, :],
                AT16aF[0:KAUG, w * 512 : (w + 1) * 512],
                start=True,
                stop=True,
            )
            nc.tensor.matmul(
                gw[:, 512:1024],
                CT16b[64 : 64 + KAUG, c, :],
                AT16bF[64 : 64 + KAUG, w * 512 : (w + 1) * 512],
                start=True,
                stop=True,
            )
            if s in SCHR:
                # high priority: the static DVE queue order must place this
                # BEFORE the nearby tree ops, or the PSUM slot frees late
                # and mm1(s+2) stalls the ACT pipeline
                with tc.high_priority(offset=60):
                    nc.vector.tensor_scalar(
                        out=K2i[:, s, :],
                        in0=gw,
                        scalar1=A_schr,
                        scalar2=B_schr,
                        op0=ALU.mult,
                        op1=ALU.add,
                    )
            else:
                nc.scalar.activation(
                    K2r[:, s, :], gw, ACTF.Exp, bias=0.0, scale=scale
                )
            if s % GRP == GRP - 1 and s < GRP * (TOT // GRP):
                grp = s // GRP
                r38t = emit_reduce(WIN * (GRP * grp), GRP)
                for i in range(0, GRP, 2):
                    pending.append((r38t, GRP * grp, i, GRP))
            if s == 61:
                # tail pair reduces while windows 62-63 still exp
                pending.append((emit_reduce(WIN * 60, 2), 60, 0, 2))
            if s == 62:
                # single so the post-loop serial chain is as short as possible
                pending.append((emit_reduce(WIN * 62, 1), 62, 0, 1))
            if (
                pending
                and s >= MM2_START
                and pending[0][1] + pending[0][3] + 2 <= s
            ):
                mm2_done = emit_mm2(pending.pop(0))
            for fn in sched[s]:
                fn()
        # tail: final window
        pending.append((emit_reduce(WIN * 63, 1), 63, 0, 1))
        for entry in pending:
            mm2_done = emit_mm2(entry)
        assert mm2_done == TOT

        # ---- epilogue: T = reduce(S); the tiny normalization/transpose
        # runs on the host (it is part of kernel(), not the reference).
        Tred = const.tile([DY + 1, BPC], FP32)
        nc.vector.tensor_reduce(
            out=Tred.rearrange("p (t o) -> p t o", o=1),
            in_=S[0 : DY + 1, :].rearrange("p (t f) -> p t f", f=4),
            axis=AX.X,
            op=ALU.add,
        )
        nc.sync.dma_start(out=out_d, in_=Tred)




def build_program(scale):
    nc = bacc.Bacc(
        "TRN2",
        target_bir_lowering=False,
        debug=False,
        enable_asserts=False,
        num_devices=NCORES,
    )
    inp = nc.dram_tensor("inputs", [BPC, N, DX], FP32, kind="ExternalInput").ap()
    cx = nc.dram_tensor("c_x", [M, DX], FP32, kind="ExternalInput").ap()
    cy = nc.dram_tensor("c_y", [M, DY], FP32, kind="ExternalInput").ap()
    cw = nc.dram_tensor("comp_w", [M], FP32, kind="ExternalInput").ap()
    out = nc.dram_tensor("out", [DY + 1, BPC], FP32, kind="ExternalOutput").ap()
    with tile.TileContext(nc) as tc:
        _body(tc, inp, cx, cy, cw, out, scale)
    nc.compile()
    return nc


_PROGRAM_CACHE: dict = {}


def _get_program(scale):
    nc = _PROGRAM_CACHE.get(scale)
    if nc is None:
        nc = build_program(scale)
        _PROGRAM_CACHE[scale] = nc
    return nc


def make_in_maps(inputs, c_x, c_y, comp_w):
    shards = np.ascontiguousarray(inputs.reshape(NCORES, BPC, N, DX))
    return [
        {
            "inputs": shards[i],
            "c_x": np.ascontiguousarray(c_x),
            "c_y": np.ascontiguousarray(c_y),
            "comp_w": np.ascontiguousarray(comp_w),
        }
        for i in range(NCORES)
    ]


def scale_from_sigma(sigma) -> float:
    s = max(float(np.asarray(sigma, dtype=np.float64)), MIN_SIGMA)
    return float(2.0 / (s * s))


def kernel(inputs, sigma, c_x, c_y, comp_w, _run_kwargs=None):
    nc = _get_program(scale_from_sigma(sigma))
    in_maps = make_in_maps(inputs, c_x, c_y, comp_w)
    res = run_bass_kernel_spmd(
        nc, in_maps, core_ids=list(range(NCORES)), **(_run_kwargs or {})
    )
    # T is [11, BPC] per core: rows 0-9 = unnormalized probs, row 10 = the
    # normalizer.  Finish the division + transpose here (tiny).
    T = np.stack([res.results[i]["out"] for i in range(NCORES)])  # [NC, 11, BPC]
    out = (T[:, :DY, :] / T[:, DY : DY + 1, :]).transpose(0, 2, 1)
    return np.ascontiguousarray(out.reshape(BS, DY)).astype(np.float32)
